# revision 1
# baseline (speedup 1.0000x reference)
"""GAT node-level layer on 8 TRN2 NeuronCores.

Strategy: destination-sharded edge processing.
 - Host (index-only preprocessing): sort edges by dst, shard by dst range
   (6250 nodes per core), window = 128 consecutive dst, chunk = 128 edges.
   All per-core structures padded to a uniform CAP so one SPMD program
   serves all cores.
 - Device phase 1: z_aug = h_shard @ [W.T | W.T a_src | W.T a_dst]
   (one matmul chain per 128-node tile); AllGather z (bf16) + s (f32);
   q stays local (only gathered by local dst).
 - Device phase 2: per window, indirect-gather z rows by src (256B rows),
   s by src, q by dst; e = leaky_relu(s+q); exp without max-subtraction
   (shift cancels in softmax; |e| <~ 30 so fp32 exp is safe); selection
   matrix S[e,d] = (seg_rel==d) * exp_e built in ONE fused tensor_scalar;
   PE accumulates out[d,:129] = sum_chunks S.T @ [z_row | 1] in PSUM
   (col 128 = softmax denominator); normalize rows by 1/denom; write out.
No scatter anywhere; no inter-core traffic except the z/s AllGather.
"""

import sys

if "/opt/trn_rl_repo" not in sys.path:
    sys.path.insert(0, "/opt/trn_rl_repo")

from contextlib import ExitStack

import numpy as np

from concourse import bacc, bass, mybir, tile
from concourse.masks import make_identity

N_NODES = 50000
N_EDGES = 800000
D_IN = 256
D_OUT = 128
CORES = 8
P = 128

F32 = mybir.dt.float32
BF16 = mybir.dt.bfloat16
I32 = mybir.dt.int32

_PROGRAM_CACHE: dict = {}


# ---------------------------------------------------------------- host prep
def preprocess_indices(src, dst, n_nodes=N_NODES, cores=CORES):
    """Sort edges by dst, shard by dst range, build padded per-core index
    arrays [wpc, 128, cap]. Integer-only work."""
    shard = n_nodes // cores
    wpc = (shard + P - 1) // P
    src = np.asarray(src).astype(np.int64)
    dst = np.asarray(dst).astype(np.int64)

    order = np.argsort(dst, kind="stable")
    ds = dst[order]
    ss = src[order]
    bounds = np.searchsorted(ds, np.arange(cores + 1) * shard)

    per_core = []
    cap = 1
    for c in range(cores):
        lo, hi = int(bounds[c]), int(bounds[c + 1])
        dloc = ds[lo:hi] - c * shard
        s_c = ss[lo:hi]
        w = dloc >> 7
        counts = np.bincount(w, minlength=wpc)
        cap = max(cap, int((counts.max() + P - 1) // P))
        per_core.append((dloc, s_c, w, counts))

    arrs = []
    for c in range(cores):
        dloc, s_c, w, counts = per_core[c]
        starts = np.zeros(wpc, np.int64)
        starts[1:] = np.cumsum(counts)[:-1]
        pos_in_w = np.arange(len(dloc), dtype=np.int64) - starts[w]
        chunk = (pos_in_w >> 7).astype(np.int64)
        epos = (pos_in_w & 127).astype(np.int64)
        src_idx = np.zeros((wpc, P, cap), np.int32)
        q_idx = np.zeros((wpc, P, cap), np.int32)
        seg_rel = np.full((wpc, P, cap), -1.0, np.float32)
        src_idx[w, epos, chunk] = s_c
        q_idx[w, epos, chunk] = dloc
        seg_rel[w, epos, chunk] = (dloc & 127).astype(np.float32)
        arrs.append({"src_idx": src_idx, "q_idx": q_idx, "seg_rel": seg_rel})
    return cap, arrs


# ---------------------------------------------------------------- program
def build_program(cap, n_nodes=N_NODES, d_in=D_IN, d_out=D_OUT, cores=CORES):
    shard = n_nodes // cores
    wpc = (shard + P - 1) // P
    kc_n = d_in // P  # k-chunks of the input dim

    nc = bacc.Bacc(None, target_bir_lowering=False, debug=False)

    h_t = nc.dram_tensor("h_t", [d_in, shard], F32, kind="ExternalInput")
    w_d = nc.dram_tensor("W", [d_out, d_in], F32, kind="ExternalInput")
    a_d = nc.dram_tensor("a", [2 * d_out, 1], F32, kind="ExternalInput")
    src_d = nc.dram_tensor("src_idx", [wpc, P, cap], I32, kind="ExternalInput")
    qid_d = nc.dram_tensor("q_idx", [wpc, P, cap], I32, kind="ExternalInput")
    seg_d = nc.dram_tensor("seg_rel", [wpc, P, cap], F32, kind="ExternalInput")
    out_d = nc.dram_tensor("out", [shard, d_out], F32, kind="ExternalOutput")

    rg = [list(range(cores))]

    with tile.TileContext(nc) as tc:
        with ExitStack() as ctx:
            dram = ctx.enter_context(tc.tile_pool(name="dram", bufs=1, space="DRAM"))
            z_bounce = dram.tile([shard, d_out], BF16)
            s_bounce = dram.tile([shard, 1], F32)
            z_full = dram.tile([n_nodes, d_out], BF16)
            s_full = dram.tile([n_nodes, 1], F32)
            q_loc = dram.tile([shard, 1], F32)

            const = ctx.enter_context(tc.tile_pool(name="const", bufs=1))

            # ---- constants
            identity = const.tile([P, P], F32)
            make_identity(nc, identity[:])
            iota_i = const.tile([P, P], I32)
            nc.gpsimd.iota(iota_i[:], pattern=[[1, P]], base=0, channel_multiplier=0)
            iota_bf = const.tile([P, P], BF16)
            nc.vector.tensor_copy(iota_bf[:], iota_i[:])
            ones_col = const.tile([P, 1], BF16)
            nc.gpsimd.memset(ones_col[:], 1.0)

            w_sb = const.tile([P, d_in], F32)
            nc.sync.dma_start(out=w_sb[:], in_=w_d[:, :])
            a_sb = const.tile([P, 2], F32)
            nc.sync.dma_start(out=a_sb[:, 0:1], in_=a_d[0:P, :])
            nc.sync.dma_start(out=a_sb[:, 1:2], in_=a_d[P : 2 * P, :])

            # W_aug_T[kc] = [W.T chunk | v_src chunk | v_dst chunk]  (bf16)
            ctx1 = ctx.enter_context(ExitStack())
            psum = ctx1.enter_context(tc.tile_pool(name="psum", bufs=2, space="PSUM"))
            waug = const.tile([P, kc_n, d_out + 2], BF16)
            for kc in range(kc_n):
                ksl = slice(kc * P, (kc + 1) * P)
                pt = psum.tile([P, P], F32, tag="pt")
                nc.tensor.transpose(pt[:], w_sb[:, ksl], identity[:])
                nc.vector.tensor_copy(waug[:, kc, 0:d_out], pt[:])
                pv = psum.tile([P, 2], F32, tag="pv")
                nc.tensor.matmul(
                    out=pv[:, 0:1], lhsT=w_sb[:, ksl], rhs=a_sb[:, 0:1],
                    start=True, stop=True,
                )
                nc.tensor.matmul(
                    out=pv[:, 1:2], lhsT=w_sb[:, ksl], rhs=a_sb[:, 1:2],
                    start=True, stop=True,
                )
                nc.vector.tensor_copy(waug[:, kc, d_out : d_out + 2], pv[:])

            # ---- phase 1: z_aug = h_shard @ W_aug, write z/s/q, AllGather
            h_sb = const.tile([P, kc_n, shard], BF16)
            for kc in range(kc_n):
                # SWDGE cast f32 -> bf16 during DMA
                nc.gpsimd.dma_start(
                    out=h_sb[:, kc, :], in_=h_t[kc * P : (kc + 1) * P, :]
                )

            zq = ctx.enter_context(tc.tile_pool(name="zq", bufs=3))
            for nt in range(wpc):
                n0 = nt * P
                rows = min(P, shard - n0)
                pz = psum.tile([P, d_out + 2], F32, tag="pz")
                for kc in range(kc_n):
                    nc.tensor.matmul(
                        out=pz[0:rows, :],
                        lhsT=h_sb[:, kc, n0 : n0 + rows],
                        rhs=waug[:, kc, :],
                        start=(kc == 0),
                        stop=(kc == kc_n - 1),
                    )
                zt = zq.tile([P, d_out], BF16, tag="zt")
                nc.vector.tensor_copy(zt[0:rows, :], pz[0:rows, 0:d_out])
                sq = zq.tile([P, 2], F32, tag="sq")
                nc.vector.tensor_copy(sq[0:rows, :], pz[0:rows, d_out : d_out + 2])
                nc.sync.dma_start(out=z_bounce[n0 : n0 + rows, :], in_=zt[0:rows, :])
                nc.sync.dma_start(out=s_bounce[n0 : n0 + rows, :], in_=sq[0:rows, 0:1])
                nc.sync.dma_start(out=q_loc[n0 : n0 + rows, :], in_=sq[0:rows, 1:2])

            ctx1.close()
            nc.gpsimd.collective_compute(
                "AllGather",
                mybir.AluOpType.bypass,
                replica_groups=rg,
                ins=[z_bounce[:, :]],
                outs=[z_full[:, :]],
            )
            nc.gpsimd.collective_compute(
                "AllGather",
                mybir.AluOpType.bypass,
                replica_groups=rg,
                ins=[s_bounce[:, :]],
                outs=[s_full[:, :]],
            )

            # ---- phase 2: per dst-window edge processing
            ep = ctx.enter_context(tc.tile_pool(name="ep", bufs=3))
            gp = ctx.enter_context(tc.tile_pool(name="gp", bufs=3))
            sp = ctx.enter_context(tc.tile_pool(name="sp", bufs=4))
            op = ctx.enter_context(tc.tile_pool(name="op", bufs=2))
            psum2 = ctx.enter_context(
                tc.tile_pool(name="psum2", bufs=2, space="PSUM")
            )

            for w in range(wpc):
                n0 = w * P
                rows = min(P, shard - n0)

                si = ep.tile([P, cap], I32, tag="si")
                qi = ep.tile([P, cap], I32, tag="qi")
                sg = ep.tile([P, cap], F32, tag="sg")
                nc.sync.dma_start(out=si[:], in_=src_d[w, :, :])
                nc.sync.dma_start(out=qi[:], in_=qid_d[w, :, :])
                nc.sync.dma_start(out=sg[:], in_=seg_d[w, :, :])

                g = gp.tile([P, cap, d_out], BF16, tag="g")
                sv = ep.tile([P, cap], F32, tag="sv")
                qv = ep.tile([P, cap], F32, tag="qv")
                for c in range(cap):
                    nc.gpsimd.indirect_dma_start(
                        out=g[:, c, :],
                        out_offset=None,
                        in_=z_full[:, :],
                        in_offset=bass.IndirectOffsetOnAxis(
                            ap=si[:, c : c + 1], axis=0
                        ),
                    )
                    nc.gpsimd.indirect_dma_start(
                        out=sv[:, c : c + 1],
                        out_offset=None,
                        in_=s_full[:, :],
                        in_offset=bass.IndirectOffsetOnAxis(
                            ap=si[:, c : c + 1], axis=0
                        ),
                    )
                    nc.gpsimd.indirect_dma_start(
                        out=qv[:, c : c + 1],
                        out_offset=None,
                        in_=q_loc[:, :],
                        in_offset=bass.IndirectOffsetOnAxis(
                            ap=qi[:, c : c + 1], axis=0
                        ),
                    )

                x = ep.tile([P, cap], F32, tag="x")
                nc.vector.tensor_tensor(
                    out=x[:], in0=sv[:], in1=qv[:], op=mybir.AluOpType.add
                )
                x2 = ep.tile([P, cap], F32, tag="x2")
                nc.scalar.activation(
                    out=x2[:], in_=x[:],
                    func=mybir.ActivationFunctionType.Copy, scale=0.01,
                )
                xm = ep.tile([P, cap], F32, tag="xm")
                nc.vector.tensor_tensor(
                    out=xm[:], in0=x[:], in1=x2[:], op=mybir.AluOpType.max
                )
                ex = ep.tile([P, cap], F32, tag="ex")
                nc.scalar.activation(
                    out=ex[:], in_=xm[:], func=mybir.ActivationFunctionType.Exp
                )

                po = psum2.tile([P, d_out], F32, tag="po")
                pod = psum2.tile([P, 1], F32, tag="pod")
                for c in range(cap):
                    s_sel = sp.tile([P, P], BF16, tag="s_sel")
                    nc.vector.tensor_scalar(
                        out=s_sel[:],
                        in0=iota_bf[:],
                        scalar1=sg[:, c : c + 1],
                        scalar2=ex[:, c : c + 1],
                        op0=mybir.AluOpType.is_equal,
                        op1=mybir.AluOpType.mult,
                    )
                    nc.tensor.matmul(
                        out=po[:],
                        lhsT=s_sel[:],
                        rhs=g[:, c, :],
                        start=(c == 0),
                        stop=(c == cap - 1),
                    )
                    nc.tensor.matmul(
                        out=pod[:, 0:1],
                        lhsT=s_sel[:],
                        rhs=ones_col[:],
                        start=(c == 0),
                        stop=(c == cap - 1),
                    )

                den = ep.tile([P, 1], F32, tag="den")
                nc.scalar.activation(
                    out=den[:], in_=pod[:],
                    func=mybir.ActivationFunctionType.Copy, bias=1e-6,
                )
                rec = ep.tile([P, 1], F32, tag="rec")
                nc.vector.reciprocal(rec[:], den[:])
                ot = op.tile([P, d_out], F32, tag="ot")
                nc.scalar.activation(
                    out=ot[:], in_=po[:],
                    func=mybir.ActivationFunctionType.Copy, scale=rec[:, 0:1],
                )
                nc.sync.dma_start(out=out_d[n0 : n0 + rows, :], in_=ot[0:rows, :])

    nc.compile()
    return nc


# ---------------------------------------------------------------- driver
def prepare(h, W, a, src, dst):
    """Build (cached) program + per-core in_maps from full inputs."""
    h = np.asarray(h, dtype=np.float32)
    W = np.asarray(W, dtype=np.float32)
    a = np.asarray(a, dtype=np.float32)
    n_nodes = h.shape[0]
    shard = n_nodes // CORES

    cap, arrs = preprocess_indices(src, dst, n_nodes=n_nodes)
    key = (cap, n_nodes, h.shape[1], W.shape[0])
    if key not in _PROGRAM_CACHE:
        _PROGRAM_CACHE[key] = build_program(
            cap, n_nodes=n_nodes, d_in=h.shape[1], d_out=W.shape[0]
        )
    nc = _PROGRAM_CACHE[key]

    in_maps = []
    for c in range(CORES):
        h_t_c = np.ascontiguousarray(h[c * shard : (c + 1) * shard].T)
        m = {"h_t": h_t_c, "W": W, "a": a}
        m.update(arrs[c])
        in_maps.append(m)
    return nc, in_maps


def kernel(h, W, a, src, dst):
    from concourse.bass_utils import run_bass_kernel_spmd

    nc, in_maps = prepare(h, W, a, src, dst)
    res = run_bass_kernel_spmd(nc, in_maps, core_ids=list(range(CORES)))
    outs = [res.results[c]["out"] for c in range(CORES)]
    return np.ascontiguousarray(np.concatenate(outs, axis=0).astype(np.float32))

